# revision 4
# baseline (speedup 1.0000x reference)
"""De-stationary causal attention (B=2, L=S=2048, H=8, E=64) on 8 TRN2 cores.

Sharding: the 16 (batch, head) pairs are distributed 2-per-core (cores 0-3
get batch 0, heads 0..7; cores 4-7 get batch 1). Each core runs the same
Bass program on its two pairs.

Per-pair algorithm (scores kept TRANSPOSED: s on partitions, l on free dim):
  ST[s, l]  = K^T_tile.T @ Q^T                       (PE, f32r)
  A[s, l]   = exp(ST * (0.125*tau) + 0.125*delta[s]) (ACT, fused scale+bias)
  diag tile masked with upper-triangular 0/1 mask    (DVE)
  OT[e+1, l] accumulates V_aug.T @ A over s-chunks   (PE; V_aug has a ones
              column, so row 64 of OT carries the softmax denominators)
  epilogue: OT -> SBUF -> PE transpose -> [l, 65] -> divide by sums -> out
"""

import copy
import sys

import numpy as np

try:
    import concourse.bass as bass
except ImportError:  # pragma: no cover
    sys.path.insert(0, "/opt/trn_rl_repo")
    import concourse.bass as bass

import concourse.mybir as mybir
import concourse.tile as tile
from concourse.bass_utils import run_bass_kernel_spmd
from concourse.vector_clock import ScopedClock

B, L, H, E = 2, 2048, 8, 64
N_CORES = 8
PAIRS_PER_CORE = 2
SCALE = 1.0 / np.sqrt(np.float32(E))  # 0.125

f32 = mybir.dt.float32
f32r = mybir.dt.float32r

# ---------------------------------------------------------------------------
# Walrus in this toolchain rejects >1 sync-wait per instruction. Split extra
# waits onto NoOps committed just before the instruction on the same engine.
# ---------------------------------------------------------------------------
_NOP_TEMPLATE = {}


def _make_nop(engine, name):
    if engine not in _NOP_TEMPLATE:
        tmp = bass.Bass()
        _NOP_TEMPLATE[engine] = tmp.engines[engine].nop(nofuse=True).ins
    nop = copy.copy(_NOP_TEMPLATE[engine])
    nop.name = name
    nop.engine = engine
    nop.sync_info = None
    return nop


class SplitWaitTileContext(tile.TileContext):
    _ws_counter = 0

    def _split_waits(self, inst):
        si = inst.sync_info
        if si is None or not si.on_wait or len(si.on_wait) <= 1:
            return []
        if inst.engine == mybir.EngineType.Unassigned:
            return []
        waits = list(si.on_wait)
        inst.sync_info = mybir.SyncInfo(
            on_wait=[waits[0]], on_update=list(si.on_update or [])
        )
        nops = []
        for w in waits[1:]:
            SplitWaitTileContext._ws_counter += 1
            nop = _make_nop(inst.engine, f"I-ws{SplitWaitTileContext._ws_counter}")
            nop.sync_info = mybir.SyncInfo(on_wait=[w], on_update=[])
            nops.append(nop)
        return nops

    def _commit_instruction(self, inst, lazy_reg_writes=True):
        for nop in self._split_waits(inst):
            self._add_instruction(nop)
        super()._commit_instruction(inst, lazy_reg_writes)

    def _drain_and_barrier(self, tick_clock, wait_clock):
        nc = self.nc
        probe = nc.sync.nop(nofuse=True)
        wait_clock.add_sem_waits(
            probe.ins, ScopedClock({None: tick_clock.global_clock})
        )
        waits = list(probe.ins.sync_info.on_wait or []) if probe.ins.sync_info else []
        if len(waits) > 1:
            probe.ins.sync_info.on_wait = [waits[0]]
            handles = {h.num: h for h in self.sems.allocated().values()}
            for w in waits[1:]:
                nop = nc.sync.nop(nofuse=True)
                nop.wait_op(handles[w.id], w.wait_value, "sem-ge")
        nc.sync.drain()

        nc.all_engine_barrier()
        assert self.sems is not None
        popped = nc._tile_sem_poison_stack.pop()
        assert popped is self._sem_poison
        nc.clear_and_free_semaphores(list(self.sems.allocated().values()))
        nc.all_engine_barrier()


# ---------------------------------------------------------------------------
# Program builder
# ---------------------------------------------------------------------------

def build_program(mm_dtype=f32r):
    nc = bass.Bass()
    Exp = mybir.ActivationFunctionType.Exp

    qt = nc.declare_dram_parameter("qt", [PAIRS_PER_CORE, E, L], mm_dtype, isOutput=False)
    kt = nc.declare_dram_parameter("kt", [PAIRS_PER_CORE, E, L], mm_dtype, isOutput=False)
    vv = nc.declare_dram_parameter("vv", [PAIRS_PER_CORE, L, E], mm_dtype, isOutput=False)
    bias_d = nc.declare_dram_parameter("bias_d", [128, L // 128], f32, isOutput=False)
    scale_t = nc.declare_dram_parameter("scale_t", [128, 1], f32, isOutput=False)
    mask = nc.declare_dram_parameter("mask", [128, 128], mm_dtype, isOutput=False)
    ident = nc.declare_dram_parameter("ident", [128, 128], f32, isOutput=False)
    oo = nc.declare_dram_parameter("oo", [PAIRS_PER_CORE, L, E], f32, isOutput=True)

    NT = L // 128  # 16 s-tiles / l-tiles
    NB = L // 512  # 4 OT banks

    with SplitWaitTileContext(nc) as tc:
        with (
            tc.tile_pool(name="const", bufs=1) as constp,
            tc.tile_pool(name="qk", bufs=2) as qkp,
            tc.tile_pool(name="vp", bufs=2) as vp,
            tc.tile_pool(name="ap", bufs=3) as ap_pool,
            tc.tile_pool(name="ep", bufs=2) as ep,
            tc.tile_pool(name="outp", bufs=2) as outp,
            tc.tile_pool(name="st", bufs=2, space="PSUM") as stp,
            tc.tile_pool(name="otp", bufs=1, space="PSUM") as otp,
        ):
            mask_sb = constp.tile([128, 128], mm_dtype, tag="mask")
            nc.sync.dma_start(out=mask_sb, in_=mask[:])
            ident_sb = constp.tile([128, 128], f32, tag="ident")
            nc.sync.dma_start(out=ident_sb, in_=ident[:])
            bias_sb = constp.tile([128, NT], f32, tag="bias")
            nc.sync.dma_start(out=bias_sb, in_=bias_d[:])
            scale_sb = constp.tile([128, 1], f32, tag="scale")
            nc.sync.dma_start(out=scale_sb, in_=scale_t[:])

            for pair in range(PAIRS_PER_CORE):
                qt_sb = qkp.tile([E, L], mm_dtype, tag="qt")
                nc.sync.dma_start(out=qt_sb, in_=qt[pair])
                kt_sb = qkp.tile([E, L], mm_dtype, tag="kt")
                nc.sync.dma_start(out=kt_sb, in_=kt[pair])
                # V slab [128, 16, 65]: col 64 of last dim = ones
                v_sb = vp.tile([128, NT, E + 1], mm_dtype, tag="v")
                # ones column: fill the whole slab with 1.0 first (contiguous
                # memset), then overwrite cols 0..E-1 with V
                nc.vector.memset(v_sb.bitcast(f32), 1.0)
                nc.sync.dma_start(
                    out=v_sb[:, :, 0:E],
                    in_=vv[pair].rearrange("(t p) e -> p t e", p=128),
                )

                ot_ps = [
                    otp.tile([E + 1, 512], f32, tag=f"ot{j}", name=f"ot{j}")
                    for j in range(NB)
                ]

                for si in range(NT):
                    l0 = si * 128
                    base = (l0 // 512) * 512
                    a_si = ap_pool.tile([128, L], mm_dtype, tag="A")
                    # chunks of <=1024, 512-aligned, covering [base, L)
                    cs = base
                    while cs < L:
                        ce = min(cs + 1024, L)
                        st = stp.tile([128, 1024], f32, tag="st")
                        for ss in range(cs, ce, 512):
                            se = ss + 512
                            vs = max(ss, l0)
                            nc.tensor.matmul(
                                st[:, vs - cs : se - cs],
                                kt_sb[:, l0 : l0 + 128],
                                qt_sb[:, vs:se],
                                start=True,
                                stop=True,
                            )
                        vs = max(cs, l0)
                        nc.scalar.activation(
                            out=a_si[:, vs:ce],
                            in_=st[:, vs - cs : ce - cs],
                            func=Exp,
                            bias=bias_sb[:, si : si + 1],
                            scale=scale_sb[:, 0:1],
                        )
                        cs = ce
                    # causal mask on the diagonal 128x128 block
                    nc.vector.tensor_mul(
                        a_si[:, l0 : l0 + 128], a_si[:, l0 : l0 + 128], mask_sb
                    )
                    # AV accumulation into the OT banks
                    for lj in range(l0 // 512, NB):
                        a_lo = max(512 * lj, l0)
                        a_hi = 512 * (lj + 1)
                        nc.tensor.matmul(
                            ot_ps[lj][:, a_lo - 512 * lj : 512],
                            v_sb[:, si, :],
                            a_si[:, a_lo:a_hi],
                            start=(si == 0),
                            stop=(si == min(4 * lj + 3, NT - 1)),
                        )

                # epilogue: transpose + normalize + store
                out_sb = outp.tile([128, NT, E], f32, tag="out")
                for lj in range(NB):
                    ot_sb = ep.tile([E + 1, 512], f32, tag="ot_sb")
                    nc.vector.tensor_copy(ot_sb, ot_ps[lj])
                    for c in range(4):
                        lt = 4 * lj + c  # l-tile index
                        ott = stp.tile([128, 1024], f32, tag="st")
                        nc.tensor.transpose(
                            ott[:, 0 : E + 1],
                            ot_sb[:, c * 128 : (c + 1) * 128],
                            ident_sb[0 : E + 1, 0 : E + 1],
                        )
                        recip = ep.tile([128, 1], f32, tag="recip")
                        nc.vector.reciprocal(recip, ott[:, E : E + 1])
                        nc.vector.tensor_scalar_mul(
                            out_sb[:, lt, :],
                            ott[:, 0:E],
                            recip,
                        )
                nc.sync.dma_start(
                    out=oo[pair].rearrange("(t p) e -> p t e", p=128),
                    in_=out_sb,
                )

    return nc


# ---------------------------------------------------------------------------
# Host-side sharding / unsharding
# ---------------------------------------------------------------------------

def _in_maps(queries, keys, values, tau, delta):
    mask = np.triu(np.ones((128, 128), dtype=np.float32))
    ident = np.eye(128, dtype=np.float32)
    maps = []
    for c in range(N_CORES):
        ps = [2 * c, 2 * c + 1]
        b = ps[0] // H
        hs = [p % H for p in ps]
        qt = np.ascontiguousarray(
            np.stack([queries[b, :, h, :].T for h in hs])
        ).astype(np.float32)
        kt = np.ascontiguousarray(
            np.stack([keys[b, :, h, :].T for h in hs])
        ).astype(np.float32)
        vv = np.ascontiguousarray(
            np.stack([values[b, :, h, :] for h in hs])
        ).astype(np.float32)
        bias_d = np.ascontiguousarray(
            (SCALE * delta[b]).reshape(L // 128, 128).T
        ).astype(np.float32)
        scale_t = np.full((128, 1), SCALE * tau[b, 0], dtype=np.float32)
        maps.append(
            {
                "qt": qt,
                "kt": kt,
                "vv": vv,
                "bias_d": bias_d,
                "scale_t": scale_t,
                "mask": mask,
                "ident": ident,
            }
        )
    return maps


_CACHED = {}


def run(queries, keys, values, tau, delta, trace=False, mm_dtype=f32r):
    key = str(mm_dtype)
    if key not in _CACHED:
        _CACHED[key] = build_program(mm_dtype)
    nc = _CACHED[key]
    in_maps = _in_maps(
        np.asarray(queries),
        np.asarray(keys),
        np.asarray(values),
        np.asarray(tau),
        np.asarray(delta),
    )
    res = run_bass_kernel_spmd(
        nc, in_maps, core_ids=list(range(N_CORES)), trace=trace
    )
    out = np.empty((B, L, H, E), dtype=np.float32)
    for c in range(N_CORES):
        o = res.results[c]["oo"]
        for i, p in enumerate([2 * c, 2 * c + 1]):
            out[p // H, :, p % H, :] = o[i]
    return out, res


def kernel(queries, keys, values, tau, delta):
    out, _ = run(queries, keys, values, tau, delta, trace=False)
    return out


# revision 6
# speedup vs baseline: 1.2664x; 1.2664x over previous
"""De-stationary causal attention (B=2, L=S=2048, H=8, E=64) on 8 TRN2 cores.

Sharding: the 16 (batch, head) pairs are distributed 2-per-core (cores 0-3
get batch 0, heads 0..7; cores 4-7 get batch 1). Each core runs the same
Bass program on its two pairs.

Per-pair algorithm (scores kept TRANSPOSED: s on partitions, l on free dim):
  ST[s, l]  = K^T_tile.T @ Q^T                       (PE, f32r)
  A[s, l]   = exp(ST * (0.125*tau) + 0.125*delta[s]) (ACT, fused scale+bias)
  diag tile masked with upper-triangular 0/1 mask    (DVE)
  OT[e+1, l] accumulates V_aug.T @ A over s-chunks   (PE; V_aug has a ones
              column, so row 64 of OT carries the softmax denominators)
  epilogue: OT -> SBUF -> PE transpose -> [l, 65] -> divide by sums -> out
"""

import copy
import sys

import numpy as np

try:
    import concourse.bass as bass
except ImportError:  # pragma: no cover
    sys.path.insert(0, "/opt/trn_rl_repo")
    import concourse.bass as bass

import concourse.mybir as mybir
import concourse.tile as tile
from concourse.bass_utils import run_bass_kernel_spmd
from concourse.vector_clock import ScopedClock

B, L, H, E = 2, 2048, 8, 64
N_CORES = 8
PAIRS_PER_CORE = 2
SCALE = 1.0 / np.sqrt(np.float32(E))  # 0.125

f32 = mybir.dt.float32
f32r = mybir.dt.float32r

# ---------------------------------------------------------------------------
# Walrus in this toolchain rejects >1 sync-wait per instruction. Split extra
# waits onto NoOps committed just before the instruction on the same engine.
# ---------------------------------------------------------------------------
_NOP_TEMPLATE = {}


def _make_nop(engine, name):
    if engine not in _NOP_TEMPLATE:
        tmp = bass.Bass()
        _NOP_TEMPLATE[engine] = tmp.engines[engine].nop(nofuse=True).ins
    nop = copy.copy(_NOP_TEMPLATE[engine])
    nop.name = name
    nop.engine = engine
    nop.sync_info = None
    return nop


class SplitWaitTileContext(tile.TileContext):
    _ws_counter = 0

    def _split_waits(self, inst):
        si = inst.sync_info
        if si is None or not si.on_wait or len(si.on_wait) <= 1:
            return []
        if inst.engine == mybir.EngineType.Unassigned:
            return []
        waits = list(si.on_wait)
        inst.sync_info = mybir.SyncInfo(
            on_wait=[waits[0]], on_update=list(si.on_update or [])
        )
        nops = []
        for w in waits[1:]:
            SplitWaitTileContext._ws_counter += 1
            nop = _make_nop(inst.engine, f"I-ws{SplitWaitTileContext._ws_counter}")
            nop.sync_info = mybir.SyncInfo(on_wait=[w], on_update=[])
            nops.append(nop)
        return nops

    def _commit_instruction(self, inst, lazy_reg_writes=True):
        for nop in self._split_waits(inst):
            self._add_instruction(nop)
        super()._commit_instruction(inst, lazy_reg_writes)

    def _drain_and_barrier(self, tick_clock, wait_clock):
        nc = self.nc
        probe = nc.sync.nop(nofuse=True)
        wait_clock.add_sem_waits(
            probe.ins, ScopedClock({None: tick_clock.global_clock})
        )
        waits = list(probe.ins.sync_info.on_wait or []) if probe.ins.sync_info else []
        if len(waits) > 1:
            probe.ins.sync_info.on_wait = [waits[0]]
            handles = {h.num: h for h in self.sems.allocated().values()}
            for w in waits[1:]:
                nop = nc.sync.nop(nofuse=True)
                nop.wait_op(handles[w.id], w.wait_value, "sem-ge")
        nc.sync.drain()

        nc.all_engine_barrier()
        assert self.sems is not None
        popped = nc._tile_sem_poison_stack.pop()
        assert popped is self._sem_poison
        nc.clear_and_free_semaphores(list(self.sems.allocated().values()))
        nc.all_engine_barrier()


# ---------------------------------------------------------------------------
# Program builder
# ---------------------------------------------------------------------------

def build_program(mm_dtype=f32r):
    nc = bass.Bass()
    Exp = mybir.ActivationFunctionType.Exp

    qt = nc.declare_dram_parameter("qt", [PAIRS_PER_CORE, E, L], mm_dtype, isOutput=False)
    kt = nc.declare_dram_parameter("kt", [PAIRS_PER_CORE, E, L], mm_dtype, isOutput=False)
    vv = nc.declare_dram_parameter("vv", [PAIRS_PER_CORE, L, E], mm_dtype, isOutput=False)
    bias_d = nc.declare_dram_parameter("bias_d", [128, L // 128], f32, isOutput=False)
    mask = nc.declare_dram_parameter("mask", [128, 128], mm_dtype, isOutput=False)
    ident = nc.declare_dram_parameter("ident", [128, 128], f32, isOutput=False)
    oo = nc.declare_dram_parameter("oo", [PAIRS_PER_CORE, L, E], f32, isOutput=True)

    NT = L // 128  # 16 s-tiles / l-tiles
    NB = L // 512  # 4 OT banks

    with SplitWaitTileContext(nc) as tc:
        with (
            tc.tile_pool(name="const", bufs=1) as constp,
            tc.tile_pool(name="qk", bufs=2) as qkp,
            tc.tile_pool(name="vp", bufs=2) as vp,
            tc.tile_pool(name="ap", bufs=3) as ap_pool,
            tc.tile_pool(name="ep", bufs=2) as ep,
            tc.tile_pool(name="outp", bufs=2) as outp,
            tc.tile_pool(name="st", bufs=2, space="PSUM") as stp,
            tc.tile_pool(name="otp", bufs=1, space="PSUM") as otp,
        ):
            mask_sb = constp.tile([128, 128], mm_dtype, tag="mask")
            nc.sync.dma_start(out=mask_sb, in_=mask[:])
            ident_sb = constp.tile([128, 128], f32, tag="ident")
            nc.sync.dma_start(out=ident_sb, in_=ident[:])
            bias_sb = constp.tile([128, NT], f32, tag="bias")
            nc.sync.dma_start(out=bias_sb, in_=bias_d[:])

            for pair in range(PAIRS_PER_CORE):
                qt_sb = qkp.tile([E, L], mm_dtype, tag="qt")
                nc.sync.dma_start(out=qt_sb, in_=qt[pair])
                kt_sb = qkp.tile([E, L], mm_dtype, tag="kt")
                nc.sync.dma_start(out=kt_sb, in_=kt[pair])
                # V slab [128, 16, 65]: col 64 of last dim = ones
                v_sb = vp.tile([128, NT, E + 1], mm_dtype, tag="v")
                # ones column: fill the whole slab with 1.0 first (contiguous
                # memset), then overwrite cols 0..E-1 with V
                nc.vector.memset(v_sb.bitcast(f32), 1.0)
                nc.sync.dma_start(
                    out=v_sb[:, :, 0:E],
                    in_=vv[pair].rearrange("(t p) e -> p t e", p=128),
                )

                ot_ps = [
                    otp.tile([E + 1, 512], f32, tag=f"ot{j}", name=f"ot{j}")
                    for j in range(NB)
                ]

                def emit_st(si, a_si):
                    """Score matmuls + exp + mask for s-tile si."""
                    l0 = si * 128
                    base = (l0 // 512) * 512
                    # chunks of <=1024, 512-aligned, covering [base, L)
                    cs = base
                    while cs < L:
                        ce = min(cs + 1024, L)
                        st = stp.tile([128, 1024], f32, tag="st", name="st")
                        for ss in range(cs, ce, 512):
                            se = ss + 512
                            vs = max(ss, l0)
                            nc.tensor.matmul(
                                st[:, vs - cs : se - cs],
                                kt_sb[:, l0 : l0 + 128],
                                qt_sb[:, vs:se],
                                start=True,
                                stop=True,
                            )
                        vs = max(cs, l0)
                        nc.scalar.activation(
                            out=a_si[:, vs:ce],
                            in_=st[:, vs - cs : ce - cs],
                            func=Exp,
                            bias=bias_sb[:, si : si + 1],
                            scale=1.0,
                        )
                        cs = ce
                    # causal mask on the diagonal 128x128 block
                    nc.vector.tensor_mul(
                        a_si[:, l0 : l0 + 128], a_si[:, l0 : l0 + 128], mask_sb
                    )

                def emit_av(si, a_si):
                    """AV accumulation of s-tile si into the OT banks."""
                    l0 = si * 128
                    for lj in range(l0 // 512, NB):
                        a_lo = max(512 * lj, l0)
                        a_hi = 512 * (lj + 1)
                        nc.tensor.matmul(
                            ot_ps[lj][:, a_lo - 512 * lj : 512],
                            v_sb[:, si, :],
                            a_si[:, a_lo:a_hi],
                            start=(si == 0),
                            stop=(si == min(4 * lj + 3, NT - 1)),
                        )

                # software pipeline: keep PE one s-tile ahead of the AV
                # consumer so it never stalls on ACT's exp
                a_tiles = {}
                a_tiles[0] = ap_pool.tile([128, L], mm_dtype, tag="A", name="A")
                emit_st(0, a_tiles[0])
                for si in range(1, NT):
                    a_tiles[si] = ap_pool.tile(
                        [128, L], mm_dtype, tag="A", name="A"
                    )
                    emit_st(si, a_tiles[si])
                    emit_av(si - 1, a_tiles[si - 1])
                    del a_tiles[si - 1]
                emit_av(NT - 1, a_tiles[NT - 1])

                # epilogue: transpose + normalize + store
                out_sb = outp.tile([128, NT, E], f32, tag="out")
                for lj in range(NB):
                    ot_sb = ep.tile([E + 1, 512], f32, tag="ot_sb")
                    nc.vector.tensor_copy(ot_sb, ot_ps[lj])
                    for c in range(4):
                        lt = 4 * lj + c  # l-tile index
                        ott = stp.tile([128, 1024], f32, tag="st")
                        nc.tensor.transpose(
                            ott[:, 0 : E + 1],
                            ot_sb[:, c * 128 : (c + 1) * 128],
                            ident_sb[0 : E + 1, 0 : E + 1],
                        )
                        recip = ep.tile([128, 1], f32, tag="recip")
                        nc.vector.reciprocal(recip, ott[:, E : E + 1])
                        nc.vector.tensor_scalar_mul(
                            out_sb[:, lt, :],
                            ott[:, 0:E],
                            recip,
                        )
                nc.sync.dma_start(
                    out=oo[pair].rearrange("(t p) e -> p t e", p=128),
                    in_=out_sb,
                )

    return nc


# ---------------------------------------------------------------------------
# Host-side sharding / unsharding
# ---------------------------------------------------------------------------

def _in_maps(queries, keys, values, tau, delta):
    mask = np.triu(np.ones((128, 128), dtype=np.float32))
    ident = np.eye(128, dtype=np.float32)
    maps = []
    for c in range(N_CORES):
        ps = [2 * c, 2 * c + 1]
        b = ps[0] // H
        hs = [p % H for p in ps]
        qscale = np.float32(SCALE * tau[b, 0])
        qt = np.ascontiguousarray(
            np.stack([queries[b, :, h, :].T * qscale for h in hs])
        ).astype(np.float32)
        kt = np.ascontiguousarray(
            np.stack([keys[b, :, h, :].T for h in hs])
        ).astype(np.float32)
        vv = np.ascontiguousarray(
            np.stack([values[b, :, h, :] for h in hs])
        ).astype(np.float32)
        bias_d = np.ascontiguousarray(
            (SCALE * delta[b]).reshape(L // 128, 128).T
        ).astype(np.float32)
        maps.append(
            {
                "qt": qt,
                "kt": kt,
                "vv": vv,
                "bias_d": bias_d,
                "mask": mask,
                "ident": ident,
            }
        )
    return maps


_CACHED = {}


def run(queries, keys, values, tau, delta, trace=False, mm_dtype=f32r):
    key = str(mm_dtype)
    if key not in _CACHED:
        _CACHED[key] = build_program(mm_dtype)
    nc = _CACHED[key]
    in_maps = _in_maps(
        np.asarray(queries),
        np.asarray(keys),
        np.asarray(values),
        np.asarray(tau),
        np.asarray(delta),
    )
    res = run_bass_kernel_spmd(
        nc, in_maps, core_ids=list(range(N_CORES)), trace=trace
    )
    out = np.empty((B, L, H, E), dtype=np.float32)
    for c in range(N_CORES):
        o = res.results[c]["oo"]
        for i, p in enumerate([2 * c, 2 * c + 1]):
            out[p // H, :, p % H, :] = o[i]
    return out, res


def kernel(queries, keys, values, tau, delta):
    out, _ = run(queries, keys, values, tau, delta, trace=False)
    return out


# revision 9
# speedup vs baseline: 1.4447x; 1.1408x over previous
"""De-stationary causal attention (B=2, L=S=2048, H=8, E=64) on 8 TRN2 cores.

Sharding: the 16 (batch, head) pairs are distributed 2-per-core (cores 0-3
get batch 0, heads 0..7; cores 4-7 get batch 1). Each core runs the same
Bass program on its two pairs.

Per-pair algorithm (scores kept TRANSPOSED: s on partitions, l on free dim):
  ST[s, l]  = K^T_tile.T @ Q^T                       (PE, f32r)
  A[s, l]   = exp(ST * (0.125*tau) + 0.125*delta[s]) (ACT, fused scale+bias)
  diag tile masked with upper-triangular 0/1 mask    (DVE)
  OT[e+1, l] accumulates V_aug.T @ A over s-chunks   (PE; V_aug has a ones
              column, so row 64 of OT carries the softmax denominators)
  epilogue: OT -> SBUF -> PE transpose -> [l, 65] -> divide by sums -> out
"""

import copy
import sys

import numpy as np

try:
    import concourse.bass as bass
except ImportError:  # pragma: no cover
    sys.path.insert(0, "/opt/trn_rl_repo")
    import concourse.bass as bass

import concourse.mybir as mybir
import concourse.tile as tile
from concourse.bass_utils import run_bass_kernel_spmd
from concourse.vector_clock import ScopedClock

B, L, H, E = 2, 2048, 8, 64
N_CORES = 8
PAIRS_PER_CORE = 2
SCALE = 1.0 / np.sqrt(np.float32(E))  # 0.125

f32 = mybir.dt.float32
f32r = mybir.dt.float32r

# ---------------------------------------------------------------------------
# Walrus in this toolchain rejects >1 sync-wait per instruction. Split extra
# waits onto NoOps committed just before the instruction on the same engine.
# ---------------------------------------------------------------------------
_NOP_TEMPLATE = {}


def _make_nop(engine, name):
    if engine not in _NOP_TEMPLATE:
        tmp = bass.Bass()
        _NOP_TEMPLATE[engine] = tmp.engines[engine].nop(nofuse=True).ins
    nop = copy.copy(_NOP_TEMPLATE[engine])
    nop.name = name
    nop.engine = engine
    nop.sync_info = None
    return nop


class SplitWaitTileContext(tile.TileContext):
    _ws_counter = 0

    def _split_waits(self, inst):
        si = inst.sync_info
        if si is None or not si.on_wait or len(si.on_wait) <= 1:
            return []
        if inst.engine == mybir.EngineType.Unassigned:
            return []
        waits = list(si.on_wait)
        inst.sync_info = mybir.SyncInfo(
            on_wait=[waits[0]], on_update=list(si.on_update or [])
        )
        nops = []
        for w in waits[1:]:
            SplitWaitTileContext._ws_counter += 1
            nop = _make_nop(inst.engine, f"I-ws{SplitWaitTileContext._ws_counter}")
            nop.sync_info = mybir.SyncInfo(on_wait=[w], on_update=[])
            nops.append(nop)
        return nops

    def _commit_instruction(self, inst, lazy_reg_writes=True):
        for nop in self._split_waits(inst):
            self._add_instruction(nop)
        super()._commit_instruction(inst, lazy_reg_writes)

    def _drain_and_barrier(self, tick_clock, wait_clock):
        nc = self.nc
        probe = nc.sync.nop(nofuse=True)
        wait_clock.add_sem_waits(
            probe.ins, ScopedClock({None: tick_clock.global_clock})
        )
        waits = list(probe.ins.sync_info.on_wait or []) if probe.ins.sync_info else []
        if len(waits) > 1:
            probe.ins.sync_info.on_wait = [waits[0]]
            handles = {h.num: h for h in self.sems.allocated().values()}
            for w in waits[1:]:
                nop = nc.sync.nop(nofuse=True)
                nop.wait_op(handles[w.id], w.wait_value, "sem-ge")
        nc.sync.drain()

        nc.all_engine_barrier()
        assert self.sems is not None
        popped = nc._tile_sem_poison_stack.pop()
        assert popped is self._sem_poison
        nc.clear_and_free_semaphores(list(self.sems.allocated().values()))
        nc.all_engine_barrier()


# ---------------------------------------------------------------------------
# Program builder
# ---------------------------------------------------------------------------

def build_program(mm_dtype=f32r):
    nc = bass.Bass()
    Exp = mybir.ActivationFunctionType.Exp

    qt = nc.declare_dram_parameter("qt", [PAIRS_PER_CORE, E, L], mm_dtype, isOutput=False)
    kt = nc.declare_dram_parameter("kt", [PAIRS_PER_CORE, E, L], mm_dtype, isOutput=False)
    vv = nc.declare_dram_parameter("vv", [PAIRS_PER_CORE, L, E], mm_dtype, isOutput=False)
    bias_d = nc.declare_dram_parameter("bias_d", [128, L // 128], f32, isOutput=False)
    mask = nc.declare_dram_parameter("mask", [128, 128], mm_dtype, isOutput=False)
    ident = nc.declare_dram_parameter("ident", [128, 128], f32, isOutput=False)
    oo = nc.declare_dram_parameter("oo", [PAIRS_PER_CORE, L, E], f32, isOutput=True)

    NT = L // 128  # 16 s-tiles / l-tiles
    NB = L // 512  # 4 OT banks

    with SplitWaitTileContext(nc) as tc:
        with (
            tc.tile_pool(name="const", bufs=1) as constp,
            tc.tile_pool(name="qk", bufs=2) as qkp,
            tc.tile_pool(name="vp", bufs=2) as vp,
            tc.tile_pool(name="ap", bufs=3) as ap_pool,
            tc.tile_pool(name="ep", bufs=2) as ep,
            tc.tile_pool(name="outp", bufs=2) as outp,
            tc.tile_pool(name="st", bufs=2, space="PSUM") as stp,
            tc.tile_pool(name="otp", bufs=1, space="PSUM") as otp,
        ):
            mask_sb = constp.tile([128, 128], mm_dtype, tag="mask")
            nc.sync.dma_start(out=mask_sb, in_=mask[:])
            ident_sb = constp.tile([128, 128], f32, tag="ident")
            nc.sync.dma_start(out=ident_sb, in_=ident[:])
            bias_sb = constp.tile([128, NT], f32, tag="bias")
            nc.sync.dma_start(out=bias_sb, in_=bias_d[:])

            for pair in range(PAIRS_PER_CORE):
                qt_sb = qkp.tile([E, L], mm_dtype, tag="qt")
                nc.sync.dma_start(out=qt_sb, in_=qt[pair])
                kt_sb = qkp.tile([E, L], mm_dtype, tag="kt")
                nc.sync.dma_start(out=kt_sb, in_=kt[pair])
                # V slab [128, 16, 65]: col 64 of last dim = ones
                v_sb = vp.tile([128, NT, E + 2], mm_dtype, tag="v")
                # ones column: fill the whole slab with 1.0 first (contiguous
                # memset), then overwrite cols 0..E-1 with V
                nc.vector.memset(v_sb, 1.0)
                nc.sync.dma_start(
                    out=v_sb[:, :, 0:E],
                    in_=vv[pair].rearrange("(t p) e -> p t e", p=128),
                )

                ot_ps = [
                    otp.tile([E + 1, 512], f32, tag=f"ot{j}", name=f"ot{j}")
                    for j in range(NB)
                ]

                def emit_st(si, a_si):
                    """Score matmuls + exp + mask for s-tile si."""
                    l0 = si * 128
                    base = (l0 // 512) * 512
                    # chunks of <=1024, 512-aligned, covering [base, L)
                    cs = base
                    while cs < L:
                        ce = min(cs + 1024, L)
                        st = stp.tile([128, 1024], f32, tag="st", name="st")
                        for ss in range(cs, ce, 512):
                            se = ss + 512
                            vs = max(ss, l0)
                            nc.tensor.matmul(
                                st[:, vs - cs : se - cs],
                                kt_sb[:, l0 : l0 + 128],
                                qt_sb[:, vs:se],
                                start=True,
                                stop=True,
                            )
                        vs = max(cs, l0)
                        nc.scalar.activation(
                            out=a_si[:, vs:ce],
                            in_=st[:, vs - cs : ce - cs],
                            func=Exp,
                            bias=bias_sb[:, si : si + 1],
                            scale=1.0,
                        )
                        cs = ce
                    # causal mask on the diagonal 128x128 block
                    nc.vector.tensor_mul(
                        a_si[:, l0 : l0 + 128], a_si[:, l0 : l0 + 128], mask_sb
                    )

                def emit_av(si, a_si):
                    """AV accumulation of s-tile si into the OT banks."""
                    l0 = si * 128
                    for lj in range(l0 // 512, NB):
                        a_lo = max(512 * lj, l0)
                        a_hi = 512 * (lj + 1)
                        nc.tensor.matmul(
                            ot_ps[lj][:, a_lo - 512 * lj : 512],
                            v_sb[:, si, 0 : E + 1],
                            a_si[:, a_lo:a_hi],
                            start=(si == 0),
                            stop=(si == min(4 * lj + 3, NT - 1)),
                        )

                # software pipeline: keep PE one s-tile ahead of the AV
                # consumer so it never stalls on ACT's exp
                a_tiles = {}
                a_tiles[0] = ap_pool.tile([128, L], mm_dtype, tag="A", name="A")
                emit_st(0, a_tiles[0])
                for si in range(1, NT):
                    a_tiles[si] = ap_pool.tile(
                        [128, L], mm_dtype, tag="A", name="A"
                    )
                    emit_st(si, a_tiles[si])
                    emit_av(si - 1, a_tiles[si - 1])
                    del a_tiles[si - 1]
                emit_av(NT - 1, a_tiles[NT - 1])

                # epilogue: transpose + normalize + store
                out_sb = outp.tile([128, NT, E], f32, tag="out")
                for lj in range(NB):
                    ot_sb = ep.tile([E + 1, 512], f32, tag="ot_sb")
                    nc.vector.tensor_copy(ot_sb, ot_ps[lj])
                    for c in range(4):
                        lt = 4 * lj + c  # l-tile index
                        ott = stp.tile([128, 1024], f32, tag="st")
                        nc.tensor.transpose(
                            ott[:, 0 : E + 1],
                            ot_sb[:, c * 128 : (c + 1) * 128],
                            ident_sb[0 : E + 1, 0 : E + 1],
                        )
                        recip = ep.tile([128, 1], f32, tag="recip")
                        nc.vector.reciprocal(recip, ott[:, E : E + 1])
                        nc.vector.tensor_scalar_mul(
                            out_sb[:, lt, :],
                            ott[:, 0:E],
                            recip,
                        )
                nc.sync.dma_start(
                    out=oo[pair].rearrange("(t p) e -> p t e", p=128),
                    in_=out_sb,
                )

    return nc


# ---------------------------------------------------------------------------
# Host-side sharding / unsharding
# ---------------------------------------------------------------------------

def _in_maps(queries, keys, values, tau, delta, mm_dtype=f32r):
    np_mm = mybir.dt.np(mm_dtype)
    mask = np.triu(np.ones((128, 128), dtype=np.float32)).astype(np_mm)
    ident = np.eye(128, dtype=np.float32)
    maps = []
    for c in range(N_CORES):
        ps = [2 * c, 2 * c + 1]
        b = ps[0] // H
        hs = [p % H for p in ps]
        qscale = np.float32(SCALE * tau[b, 0])
        qt = np.ascontiguousarray(
            np.stack([queries[b, :, h, :].T * qscale for h in hs])
        ).astype(np_mm)
        kt = np.ascontiguousarray(
            np.stack([keys[b, :, h, :].T for h in hs])
        ).astype(np_mm)
        vv = np.ascontiguousarray(
            np.stack([values[b, :, h, :] for h in hs])
        ).astype(np_mm)
        bias_d = np.ascontiguousarray(
            (SCALE * delta[b]).reshape(L // 128, 128).T
        ).astype(np.float32)
        maps.append(
            {
                "qt": qt,
                "kt": kt,
                "vv": vv,
                "bias_d": bias_d,
                "mask": mask,
                "ident": ident,
            }
        )
    return maps


_CACHED = {}


def run(queries, keys, values, tau, delta, trace=False, mm_dtype=f32r):
    key = str(mm_dtype)
    if key not in _CACHED:
        _CACHED[key] = build_program(mm_dtype)
    nc = _CACHED[key]
    in_maps = _in_maps(
        np.asarray(queries),
        np.asarray(keys),
        np.asarray(values),
        np.asarray(tau),
        np.asarray(delta),
        mm_dtype=mm_dtype,
    )
    res = run_bass_kernel_spmd(
        nc, in_maps, core_ids=list(range(N_CORES)), trace=trace
    )
    out = np.empty((B, L, H, E), dtype=np.float32)
    for c in range(N_CORES):
        o = res.results[c]["oo"]
        for i, p in enumerate([2 * c, 2 * c + 1]):
            out[p // H, :, p % H, :] = o[i]
    return out, res


def kernel(queries, keys, values, tau, delta):
    out, _ = run(queries, keys, values, tau, delta, trace=False)
    return out
